# revision 8
# baseline (speedup 1.0000x reference)
"""Trainium2 Bass kernel for AlphaFold-style gated attention.

Reference math (B=4, N=1024, C=512, H=8, CH=64):
    q = (q_x @ Wq) / 8 ; k = kv_x @ Wk ; v = kv_x @ Wv
    s = q k^T + bias_mask[b,k] + bias_pair[h,q,k]
    a = softmax_k(s) ; o = a @ v
    g = sigmoid(q_x @ Wg + bg)
    out = (o*g) @ Wo + bo

Sharding: 8 cores = (batch b in 0..3) x (q-half qh in 0..1). Zero collectives.

Device-side trick sheet:
  - All activations kept transposed ([feat on partitions, rows on free]);
    host pre-transposes inputs, so no on-chip transposes at all.
  - exp without max subtraction (scores are O(5), fp32 exp is safe).
  - bias_mask folded into v: v' = v * exp(mask)[k]  (per-partition scale).
  - bias_pair folded as host-precomputed exp(pair)^T, multiplied into exp(s).
  - softmax denominator = extra ones*em column in v -> free row in AV matmul.
  - 1/d broadcast across partitions via K=1 outer-product matmul into PSUM.
"""

import sys

import numpy as np

if "/opt/trn_rl_repo" not in sys.path:
    sys.path.insert(0, "/opt/trn_rl_repo")

import ml_dtypes

import concourse.bass as bass  # noqa: F401
import concourse.tile as tile
from concourse import bacc, mybir
from concourse.bass_utils import run_bass_kernel_spmd

B, N, C, H, CH = 4, 1024, 512, 8, 64
R = 512          # q rows per core
KC = N // 128    # 8 k chunks of 128
CC = C // 128    # 4 feature chunks of 128
F32 = mybir.dt.float32
BF16 = mybir.dt.bfloat16
BF = ml_dtypes.bfloat16


def build(finalize=True):
    nc = bacc.Bacc("TRN2", target_bir_lowering=False, debug=False)

    qxt = nc.dram_tensor("qxt", [C, R], BF16, kind="ExternalInput").ap()
    kvt = nc.dram_tensor("kvt", [C, N], BF16, kind="ExternalInput").ap()
    kvem = nc.dram_tensor("kvem", [C, N], BF16, kind="ExternalInput").ap()
    emb = nc.dram_tensor("emb", [128, KC], BF16, kind="ExternalInput").ap()
    pairt = nc.dram_tensor("pairt", [H, N, R], BF16, kind="ExternalInput").ap()
    wq = nc.dram_tensor("wq", [C, C], BF16, kind="ExternalInput").ap()
    wk = nc.dram_tensor("wk", [C, C], BF16, kind="ExternalInput").ap()
    wv = nc.dram_tensor("wv", [C, C], BF16, kind="ExternalInput").ap()
    wg = nc.dram_tensor("wg", [C, C], BF16, kind="ExternalInput").ap()
    wo = nc.dram_tensor("wo", [C, C], BF16, kind="ExternalInput").ap()
    bgr = nc.dram_tensor("bgr", [128, CC], F32, kind="ExternalInput").ap()
    bor = nc.dram_tensor("bor", [128, CC], F32, kind="ExternalInput").ap()
    out = nc.dram_tensor("out", [C, R], F32, kind="ExternalOutput").ap()

    with tile.TileContext(nc) as tc:
        _body(tc, qxt, kvt, kvem, emb, pairt, wq, wk, wv, wg, wo, bgr, bor, out)
    if finalize:
        nc.finalize()
    return nc


def _body(tc, qxt, kvt, kvem, emb, pairt, wq, wk, wv, wg, wo, bgr, bor, out):
    nc = tc.nc
    Exp = mybir.ActivationFunctionType.Exp
    Sigmoid = mybir.ActivationFunctionType.Sigmoid
    Ident = mybir.ActivationFunctionType.Identity

    with (
        tc.tile_pool(name="keep", bufs=1) as keep,
        tc.tile_pool(name="sb", bufs=3) as sb,
        tc.tile_pool(name="pairp", bufs=2) as pairp,
        tc.tile_pool(name="dp", bufs=3) as dp,
        tc.tile_pool(name="outp", bufs=2) as outp,
        tc.tile_pool(name="psA", bufs=2, space="PSUM") as psA,
        tc.tile_pool(name="psS", bufs=2, space="PSUM") as psS,
        tc.tile_pool(name="psO", bufs=2, space="PSUM") as psO,
        tc.tile_pool(name="psD", bufs=1, space="PSUM") as psD,
        tc.tile_pool(name="psT", bufs=1, space="PSUM") as psT,
    ):
        # ---- load constants / weights ----
        w_sb = {}
        for name, ap in (("wq", wq), ("wk", wk), ("wv", wv), ("wg", wg),
                         ("wo", wo)):
            t = keep.tile([128, CC, C], BF16, tag=name)
            nc.sync.dma_start(out=t, in_=ap.rearrange("(cc p) o -> p cc o", p=128))
            w_sb[name] = t

        qxt_sb = keep.tile([128, CC, R], BF16, tag="qxt")
        nc.sync.dma_start(out=qxt_sb, in_=qxt.rearrange("(cc p) r -> p cc r", p=128))
        kvt_sb = keep.tile([128, CC, N], BF16, tag="kvt")
        nc.sync.dma_start(out=kvt_sb, in_=kvt.rearrange("(cc p) n -> p cc n", p=128))

        kvem_sb = keep.tile([128, CC, N], BF16, tag="kvem")
        nc.sync.dma_start(out=kvem_sb, in_=kvem.rearrange("(cc p) n -> p cc n", p=128))

        bgr_sb = keep.tile([128, CC], F32, tag="bgr")
        nc.sync.dma_start(out=bgr_sb, in_=bgr)
        bor_sb = keep.tile([128, CC], F32, tag="bor")
        nc.sync.dma_start(out=bor_sb, in_=bor)

        ones_64 = keep.tile([1, 64], BF16, tag="ones_64")
        nc.vector.memset(ones_64, 1.0)

        # Touch ops: advance each engine's vector clock past the input-DMA
        # semaphore lanes so no compute instruction needs >2 sync waits
        # (walrus ISA wait-slot limit).
        tps = psT.tile([1, 4], F32, tag="touch")
        nc.tensor.matmul(tps[0:1, 0:1], w_sb["wq"][0:1, 0, 0:1],
                         qxt_sb[0:1, 0, 0:1], start=True, stop=True)
        nc.tensor.matmul(tps[0:1, 1:2], w_sb["wk"][0:1, 0, 0:1],
                         kvt_sb[0:1, 0, 0:1], start=True, stop=True)
        nc.tensor.matmul(tps[0:1, 2:3], w_sb["wv"][0:1, 0, 0:1],
                         kvem_sb[0:1, 0, 0:1], start=True, stop=True)
        nc.tensor.matmul(tps[0:1, 3:4], w_sb["wg"][0:1, 0, 0:1],
                         w_sb["wo"][0:1, 0, 0:1], start=True, stop=True)
        scr = keep.tile([1, 2], F32, tag="scr")
        nc.scalar.activation(scr[0:1, 0:1], bgr_sb[0:1, 0:1], Ident)
        nc.scalar.activation(scr[0:1, 1:2], bor_sb[0:1, 0:1], Ident)
        scr2 = keep.tile([1, 1], BF16, tag="scr2")

        # ---- projections: qT, kT (bf16), gT (f32) in transposed layout ----
        qT = keep.tile([128, CC, R], BF16, tag="qT")
        kT = keep.tile([128, CC, N], BF16, tag="kT")
        gT = keep.tile([128, CC, R], F32, tag="gT")

        for cc in range(CC):
            ps = psA.tile([128, R], F32, tag="proj")
            for ci in range(CC):
                nc.tensor.matmul(
                    ps, w_sb["wq"][:, ci, cc * 128:(cc + 1) * 128],
                    qxt_sb[:, ci, :], start=(ci == 0), stop=(ci == CC - 1))
            nc.vector.tensor_copy(qT[:, cc, :], ps)

            for nh in range(2):
                ps2 = psA.tile([128, R], F32, tag="proj")
                for ci in range(CC):
                    nc.tensor.matmul(
                        ps2, w_sb["wk"][:, ci, cc * 128:(cc + 1) * 128],
                        kvt_sb[:, ci, nh * 512:(nh + 1) * 512],
                        start=(ci == 0), stop=(ci == CC - 1))
                nc.vector.tensor_copy(kT[:, cc, nh * 512:(nh + 1) * 512], ps2)

            ps3 = psA.tile([128, R], F32, tag="proj")
            for ci in range(CC):
                nc.tensor.matmul(
                    ps3, w_sb["wg"][:, ci, cc * 128:(cc + 1) * 128],
                    qxt_sb[:, ci, :], start=(ci == 0), stop=(ci == CC - 1))
            nc.scalar.activation(gT[:, cc, :], ps3, Sigmoid,
                                 bias=bgr_sb[:, cc:cc + 1])

        # ---- v natural layout, scaled by em, with ones*em column ----
        vS = keep.tile([128, KC, H, 65], BF16, tag="vS")
        for kc in range(KC):
            ps = psA.tile([128, R], F32, tag="proj")
            for ci in range(CC):
                nc.tensor.matmul(
                    ps, kvem_sb[:, ci, kc * 128:(kc + 1) * 128],
                    w_sb["wv"][:, ci, :], start=(ci == 0), stop=(ci == CC - 1))
            nc.vector.tensor_copy(vS[:, kc, :, 0:64], ps)
        emb_bcast = bass.AP(tensor=emb.tensor, offset=emb.offset,
                            ap=[[KC, 128], [1, KC], [0, H]])
        nc.gpsimd.dma_start(out=vS[:, :, :, 64:65], in_=emb_bcast)

        # ---- attention per head ----
        xgT = keep.tile([128, CC, R], BF16, tag="xgT")
        for h in range(H):
            cc, po = h // 2, (h % 2) * 64
            pairt_h = pairp.tile([128, KC, R], BF16, tag="pair")
            nc.sync.dma_start(
                out=pairt_h,
                in_=pairt[h].rearrange("(kc p) r -> p kc r", p=128))

            nc.vector.tensor_copy(scr2, pairt_h[0:1, 0, 0:1])
            ov = psO.tile([65, R], F32, tag="ov")
            for kc in range(KC):
                ss = psS.tile([128, R], F32, tag="s")
                nc.tensor.matmul(
                    ss, kT[po:po + 64, cc, kc * 128:(kc + 1) * 128],
                    qT[po:po + 64, cc, :], start=True, stop=True)
                e = sb.tile([128, R], BF16, tag="e")
                nc.scalar.activation(e, ss, Exp)
                a_t = sb.tile([128, R], BF16, tag="at")
                nc.vector.tensor_mul(a_t, e, pairt_h[:, kc, :])
                nc.tensor.matmul(ov, vS[:, kc, h, :], a_t,
                                 start=(kc == 0), stop=(kc == KC - 1))

            # normalize + gate: xgT_h = (ov[0:64]/d) * gT_h
            dinv = dp.tile([1, R], BF16, tag="dinv")
            with nc.allow_low_precision(reason="1/d to bf16 for bcast matmul"):
                nc.vector.reciprocal(dinv, ov[64:65, :])
            db = psD.tile([64, R], F32, tag="db")
            nc.tensor.matmul(db, ones_64, dinv, start=True, stop=True)
            gd = dp.tile([64, R], F32, tag="gd")
            nc.vector.tensor_mul(gd, gT[po:po + 64, cc, :], db)
            nc.vector.tensor_mul(xgT[po:po + 64, cc, :], ov[0:64, :], gd)

        # ---- output projection + bias ----
        out_r = out.rearrange("(cc p) r -> cc p r", p=128)
        for cc in range(CC):
            ps = psA.tile([128, R], F32, tag="proj")
            for ci in range(CC):
                nc.tensor.matmul(
                    ps, w_sb["wo"][:, ci, cc * 128:(cc + 1) * 128],
                    xgT[:, ci, :], start=(ci == 0), stop=(ci == CC - 1))
            osb = outp.tile([128, R], F32, tag="out")
            nc.scalar.activation(osb, ps, Ident, bias=bor_sb[:, cc:cc + 1])
            nc.sync.dma_start(out=out_r[cc], in_=osb)


def prep_in_maps(q_x, kv_x, bias_mask, bias_pair, Wq, Wk, Wv, Wg, bg, Wo, bo):
    f32 = np.float32
    shared = {
        "wq": np.ascontiguousarray((Wq.astype(f32) * 0.125).astype(BF)),
        "wk": np.ascontiguousarray(Wk.astype(BF)),
        "wv": np.ascontiguousarray(Wv.astype(BF)),
        "wg": np.ascontiguousarray(Wg.astype(BF)),
        "wo": np.ascontiguousarray(Wo.astype(BF)),
        "bgr": np.ascontiguousarray(
            np.asarray(bg, f32).reshape(CC, 128).T),
        "bor": np.ascontiguousarray(
            np.asarray(bo, f32).reshape(CC, 128).T),
    }
    pair_exp_t = {}
    bp = np.asarray(bias_pair, f32)[0]  # [H, N, N] (h, q, k)
    for qh in (0, 1):
        sl = bp[:, qh * R:(qh + 1) * R, :]          # [H, R(q), N(k)]
        pair_exp_t[qh] = np.ascontiguousarray(
            np.exp(sl).transpose(0, 2, 1).astype(BF))  # [H, N(k), R(q)]

    in_maps = []
    for i in range(8):
        b, qh = i // 2, i % 2
        m = dict(shared)
        m["qxt"] = np.ascontiguousarray(
            np.asarray(q_x[b, qh * R:(qh + 1) * R, :], f32).T.astype(BF))
        m["kvt"] = np.ascontiguousarray(np.asarray(kv_x[b], f32).T.astype(BF))
        em = np.exp(np.asarray(bias_mask[b, 0, 0], f32))
        m["kvem"] = np.ascontiguousarray(
            (np.asarray(kv_x[b], f32) * em[:, None]).T.astype(BF))
        m["emb"] = np.ascontiguousarray(em.reshape(KC, 128).T.astype(BF))
        m["pairt"] = pair_exp_t[qh]
        in_maps.append(m)
    return in_maps


def assemble(results):
    out = np.empty((B, N, C), np.float32)
    for i, r in enumerate(results):
        b, qh = i // 2, i % 2
        out[b, qh * R:(qh + 1) * R, :] = np.asarray(r["out"], np.float32).T
    return out


def kernel(q_x, kv_x, bias_mask, bias_pair, Wq, Wk, Wv, Wg, bg, Wo, bo):
    nc = build()
    in_maps = prep_in_maps(q_x, kv_x, bias_mask, bias_pair,
                           Wq, Wk, Wv, Wg, bg, Wo, bo)
    res = run_bass_kernel_spmd(nc, in_maps, core_ids=list(range(8)))
    return assemble(res.results)


if __name__ == "__main__":
    nc = build()
    print("build OK")
